# revision 16
# baseline (speedup 1.0000x reference)
"""Causal attention for Trainium2, sequence-parallel over 8 NeuronCores. v3.

Differences vs kernel2 (transposed-scores):
  * Scores computed in NATURAL layout [q, keys]: 4 chained 512-col matmuls
    per 4-key-tile group (minimal PE instruction count).
  * Row-sums l come FREE from the Exp activation's accum_out (sums along
    the free/key axis in natural layout) -- no ones-matmuls.
  * Causality on the last 8 key tiles enforced with an ADDITIVE bf16 mask
    (0 / -32768) on the PSUM scores before exp (exp underflows to 0).
  * p is transposed on PE (4 identity-matmuls per group) into pT, which
    feeds the same z accumulation as kernel2.
  * Finalize is stage-split across the next q-tile's groups.
"""

import sys
from contextlib import ExitStack, nullcontext

if "/opt/trn_rl_repo" not in sys.path:
    sys.path.insert(0, "/opt/trn_rl_repo")

import numpy as np
import ml_dtypes

import concourse.bass as bass
import concourse.tile as tile
from concourse import bacc, mybir
from concourse.bass_utils import run_bass_kernel_spmd
from concourse.masks import make_identity

F32 = mybir.dt.float32
BF16 = mybir.dt.bfloat16
NPBF = ml_dtypes.bfloat16

N, D, NCORES = 8192, 512, 8
P = 128
NT = 8                 # q-tiles per core
DC = D // P            # 4 d-chunks
KT = N // P            # 64 key tiles total
MASKVAL = -32768.0     # additive causal mask; exp() underflows to exactly 0


def build(reps=1, trace_sim=False, stage="full", sbufs=2, tbufs=2, tlag=2,
          lacc=True):
    q_rows = NT * P
    nc = bacc.Bacc("TRN2", target_bir_lowering=False, debug=False,
                   num_devices=NCORES)
    xn_d = nc.dram_tensor("xn", [N, D], BF16, kind="ExternalInput").ap()
    xt_d = nc.dram_tensor("xt", [D, N], BF16, kind="ExternalInput").ap()
    xqt_d = nc.dram_tensor("xqt", [D, q_rows], BF16, kind="ExternalInput").ap()
    a_d = nc.dram_tensor("amat", [D, D], BF16, kind="ExternalInput").ap()
    wvt_d = nc.dram_tensor("wvt", [D, D], BF16, kind="ExternalInput").ap()
    v_d = nc.dram_tensor("vvec", [P, DC], F32, kind="ExternalInput").ap()
    mq_d = nc.dram_tensor("maskq", [P, NT * P], BF16, kind="ExternalInput").ap()
    mm_d = nc.dram_tensor("maskm", [P, NT * P], BF16, kind="ExternalInput").ap()
    out_d = nc.dram_tensor("out", [q_rows, D], F32, kind="ExternalOutput").ap()

    with tile.TileContext(nc, trace_sim=trace_sim) as tc, ExitStack() as st:
        consts = st.enter_context(tc.tile_pool(name="consts", bufs=1))
        big = st.enter_context(tc.tile_pool(name="big", bufs=1))
        la_p = st.enter_context(tc.tile_pool(name="lap", bufs=2))
        pn_p = st.enter_context(tc.tile_pool(name="pnp", bufs=4))
        pt_p = st.enter_context(tc.tile_pool(name="ptp", bufs=3))
        zn_p = st.enter_context(tc.tile_pool(name="znp", bufs=2))
        out_p = st.enter_context(tc.tile_pool(name="outp", bufs=2))
        ps_s = st.enter_context(tc.tile_pool(name="ps_s", bufs=sbufs, space="PSUM"))
        ps_t = st.enter_context(tc.tile_pool(name="ps_t", bufs=tbufs, space="PSUM"))
        ps_z = st.enter_context(tc.tile_pool(name="ps_z", bufs=2, space="PSUM"))
        ps_fo = st.enter_context(tc.tile_pool(name="ps_fo", bufs=2, space="PSUM"))

        loop = tc.For_i(0, reps, 1) if reps > 1 else nullcontext()
        with loop:
            ident = consts.tile([P, P], F32, tag="ident")
            make_identity(nc, ident)
            ident_b = consts.tile([P, P], BF16, tag="ident_b")
            nc.vector.tensor_copy(out=ident_b, in_=ident)

            a_sb = consts.tile([P, DC, D], BF16, tag="a_sb")
            nc.gpsimd.dma_start(out=a_sb,
                                in_=a_d.rearrange("(c p) d -> p c d", p=P))
            xqt_sb = consts.tile([P, DC, q_rows], BF16, tag="xqt")
            for qh in range(2):
                nc.gpsimd.dma_start(
                    out=xqt_sb[:, :, qh * 512:(qh + 1) * 512],
                    in_=xqt_d[:, qh * 512:(qh + 1) * 512]
                        .rearrange("(c p) n -> p c n", p=P))
            v_sb = consts.tile([P, DC], F32, tag="v_sb")
            nc.gpsimd.dma_start(out=v_sb, in_=v_d)
            mq_sb = consts.tile([P, NT * P], BF16, tag="mq")
            nc.gpsimd.dma_start(out=mq_sb, in_=mq_d if lacc else mm_d)

            # x resident in both layouts; 16 chunks, xt slice before xn slice
            # (scores consume xt earlier than z consumes xn).
            xn_sb = big.tile([P, KT, D], BF16, tag="xn")
            xt_sb = big.tile([P, DC, N], BF16, tag="xt")
            wvt_sb = consts.tile([P, DC, D], BF16, tag="wvt")
            for b in range(16):
                r0 = b * 512
                nc.gpsimd.dma_start(
                    out=xt_sb[:, :, r0:r0 + 512],
                    in_=xt_d[:, r0:r0 + 512].rearrange("(c p) n -> p c n", p=P))
                if stage != "sx":
                    nc.gpsimd.dma_start(
                        out=xn_sb[:, 4 * b:4 * b + 4, :],
                        in_=xn_d[r0:r0 + 512, :]
                            .rearrange("(t p) d -> p t d", p=P))
                if b == 0:
                    nc.gpsimd.dma_start(
                        out=wvt_sb,
                        in_=wvt_d.rearrange("(c p) d -> p c d", p=P))

            # ---- yT[dch*P+p, q] = (A.T @ xq.T)[d, q] + v[d] ----
            yT = consts.tile([P, DC, q_rows], BF16, tag="yT")
            for qh in range(q_rows // 512):
                for dch in range(DC):
                    ps = ps_fo.tile([P, 512], F32, tag="ps_fo")
                    for c in range(DC):
                        nc.tensor.matmul(ps,
                                         a_sb[:, c, dch * P:(dch + 1) * P],
                                         xqt_sb[:, c, qh * 512:(qh + 1) * 512],
                                         start=(c == 0), stop=(c == DC - 1))
                    nc.vector.tensor_scalar_add(
                        out=yT[:, dch, qh * 512:(qh + 1) * 512], in0=ps,
                        scalar1=v_sb[:, dch:dch + 1])

            linv = consts.tile([P, NT], F32, tag="linv")

            def fin_a(t, psz, la, G):
                lsum = out_p.tile([P, 1], F32, tag="lsum")
                nc.vector.reduce_sum(lsum, la[:, :G], axis=mybir.AxisListType.X)
                nc.vector.reciprocal(linv[:, t:t + 1], lsum)
                zn = zn_p.tile([P, D], F32, tag="zn")
                nc.vector.tensor_copy(out=zn, in_=psz)
                return zn

            def fin_b(zn):
                psf = ps_fo.tile([P, 512], F32, tag="ps_fo")
                for ch in range(DC):
                    nc.tensor.transpose(psf[:, ch * P:(ch + 1) * P],
                                        zn[:, ch * P:(ch + 1) * P], ident)
                return psf

            def fin_c(psf):
                znT = zn_p.tile([P, D], BF16, tag="znT")
                nc.vector.tensor_copy(out=znT, in_=psf)
                return znT

            def fin_d(t, znT):
                pso = ps_fo.tile([P, 512], F32, tag="ps_fo")
                for ch in range(DC):
                    nc.tensor.matmul(pso, znT[:, ch * P:(ch + 1) * P],
                                     wvt_sb[:, ch, :],
                                     start=(ch == 0), stop=(ch == DC - 1))
                ot = out_p.tile([P, D], F32, tag="outp")
                nc.vector.tensor_scalar_mul(out=ot, in0=pso,
                                            scalar1=linv[:, t:t + 1])
                nc.gpsimd.dma_start(out=out_d[t * P:(t + 1) * P, :], in_=ot)

            # pending finalize state machine: list of (stage_idx, t, payload)
            pending = []

            def pump_fin():
                if not pending:
                    return
                st_i, ft, payload = pending[0]
                if st_i == 0:
                    pending[0] = (1, ft, fin_a(ft, *payload))
                elif st_i == 1:
                    pending[0] = (2, ft, fin_b(payload))
                elif st_i == 2:
                    pending[0] = (3, ft, fin_c(payload))
                else:
                    fin_d(ft, payload)
                    pending.pop(0)

            for t in range(NT):
                Kt = 8 * (t + 1)
                G = Kt // 4
                psz = ps_z.tile([P, D], F32, tag="ps_z")
                la = la_p.tile([P, 16], F32, tag="la")
                tq = []   # (i, pt_nat) awaiting transpose
                zq = []   # (i, pT_sb) awaiting z-matmuls
                fin_budget = 4
                for i in range(G):
                    pss = ps_s.tile([P, 512], F32, tag="ps_s")
                    for c in range(DC):
                        nc.tensor.matmul(pss,
                                         yT[:, c, t * P:(t + 1) * P],
                                         xt_sb[:, c, 4 * i * P:(4 * i + 4) * P],
                                         start=(c == 0), stop=(c == DC - 1))
                    if lacc and i >= G - 2:
                        mh = i - (G - 2)
                        nc.vector.tensor_add(
                            out=pss, in0=pss,
                            in1=mq_sb[:, mh * 512:(mh + 1) * 512])
                    if stage in ("s", "sx"):
                        continue
                    pn = pn_p.tile([P, 512], BF16, tag="pn")
                    if lacc:
                        nc.scalar.activation(
                            out=pn, in_=pss,
                            func=mybir.ActivationFunctionType.Exp,
                            accum_out=la[:, i:i + 1])
                    else:
                        nc.scalar.activation(
                            out=pn, in_=pss,
                            func=mybir.ActivationFunctionType.Exp)
                        if i >= G - 2:
                            mh = i - (G - 2)
                            nc.vector.tensor_mul(
                                out=pn, in0=pn,
                                in1=mq_sb[:, mh * 512:(mh + 1) * 512])
                        nc.vector.reduce_sum(la[:, i:i + 1], pn,
                                             axis=mybir.AxisListType.X)
                    if stage == "se":
                        continue
                    tq.append((i, pn))
                    # lag-1: transpose group i-1 while scores of i stream
                    if len(tq) > tlag:
                        ti, tpn = tq.pop(0)
                        pst = ps_t.tile([P, 512], BF16, tag="ps_t")
                        for j in range(4):
                            nc.tensor.transpose(pst[:, j * P:(j + 1) * P],
                                                tpn[:, j * P:(j + 1) * P],
                                                ident_b)
                        ptb = pt_p.tile([P, 512], BF16, tag="pt")
                        nc.vector.tensor_copy(out=ptb, in_=pst)
                        zq.append((ti, ptb))
                    if len(zq) > 1:
                        zi, zpt = zq.pop(0)
                        for j in range(4):
                            kt = 4 * zi + j
                            nc.tensor.matmul(psz, zpt[:, j * P:(j + 1) * P],
                                             xn_sb[:, kt, :],
                                             start=(kt == 0),
                                             stop=(kt == Kt - 1))
                    if fin_budget > 0 and i >= 1:
                        pump_fin()
                        fin_budget -= 1
                if stage in ("s", "sx", "se"):
                    continue
                # drain
                while tq:
                    ti, tpn = tq.pop(0)
                    pst = ps_t.tile([P, 512], BF16, tag="ps_t")
                    for j in range(4):
                        nc.tensor.transpose(pst[:, j * P:(j + 1) * P],
                                            tpn[:, j * P:(j + 1) * P], ident_b)
                    ptb = pt_p.tile([P, 512], BF16, tag="pt")
                    nc.vector.tensor_copy(out=ptb, in_=pst)
                    zq.append((ti, ptb))
                while zq:
                    zi, zpt = zq.pop(0)
                    for j in range(4):
                        kt = 4 * zi + j
                        nc.tensor.matmul(psz, zpt[:, j * P:(j + 1) * P],
                                         xn_sb[:, kt, :],
                                         start=(kt == 0), stop=(kt == Kt - 1))
                if stage == "full":
                    pending.append((0, t, (psz, la, G)))
            if stage == "full":
                while pending:
                    pump_fin()

    nc.compile()
    return nc


def core_rows(c):
    tiles = list(range(c, KT, NCORES))
    return np.concatenate([np.arange(g * P, (g + 1) * P) for g in tiles])


def _check_causal_mask(mask):
    m = np.asarray(mask)
    assert m.shape == (N, N), f"mask shape {m.shape}"
    rng = np.random.default_rng(0)
    rows = rng.choice(N, size=64, replace=False)
    cols = np.arange(N)
    sub = m[rows]
    expect = np.where(cols[None, :] <= rows[:, None], 0.0, -1e9).astype(np.float32)
    if not np.array_equal(sub, expect):
        raise ValueError("mask is not the expected causal mask; "
                         "this kernel hardcodes causal structure")


def prepare_in_maps(x, mask, Wq, bq, Wk, bk, Wv, bv):
    x = np.asarray(x, dtype=np.float32)
    _check_causal_mask(mask)
    inv_sqrt_d = 1.0 / np.sqrt(D)
    A = (np.asarray(Wq).T.astype(np.float64) @ np.asarray(Wk).astype(np.float64)
         * inv_sqrt_d).astype(np.float32)
    wvT = np.ascontiguousarray(np.asarray(Wv).T)
    vvec = (np.asarray(Wk).T @ np.asarray(bq) * inv_sqrt_d).astype(np.float32)
    vvec = np.ascontiguousarray(vvec.reshape(DC, P).T)  # [P, DC]
    xn_b = x.astype(NPBF)
    xt_b = np.ascontiguousarray(x.T).astype(NPBF)
    a_b = A.astype(NPBF)
    wvt_b = wvT.astype(NPBF)

    qp = np.arange(P)[:, None, None]
    kl = np.arange(NT)[None, :, None]
    kp = np.arange(P)[None, None, :]
    rows = [core_rows(c) for c in range(NCORES)]
    in_maps = []
    for c in range(NCORES):
        live = (kl * P + kp <= c * P + qp)           # [qp, kl, kp]
        mq = np.where(live, 0.0, MASKVAL).astype(NPBF).reshape(P, NT * P)
        mm = live.astype(NPBF).reshape(P, NT * P)
        xqt = np.ascontiguousarray(x[rows[c]].T).astype(NPBF)
        in_maps.append({
            "xn": xn_b, "xt": xt_b, "xqt": xqt, "amat": a_b,
            "wvt": wvt_b, "vvec": vvec,
            "maskq": np.ascontiguousarray(mq),
            "maskm": np.ascontiguousarray(mm),
        })
    meta = {"rows": rows, "bv": np.asarray(bv, dtype=np.float32)}
    return in_maps, meta


_CACHED = {}


def kernel(x, mask, Wq, bq, Wk, bk, Wv, bv):
    x = np.asarray(x)
    in_maps, meta = prepare_in_maps(x, mask, Wq, bq, Wk, bk, Wv, bv)
    if "nc" not in _CACHED:
        _CACHED["nc"] = build()
    nc = _CACHED["nc"]
    res = run_bass_kernel_spmd(nc, in_maps, list(range(NCORES)))
    out = np.empty((N, D), np.float32)
    for c in range(NCORES):
        out[meta["rows"][c]] = res.results[c]["out"]
    out += meta["bv"][None, :]
    return out


# revision 17
# speedup vs baseline: 1.0733x; 1.0733x over previous
"""Causal attention for Trainium2, sequence-parallel over 8 NeuronCores. v3.

Differences vs kernel2 (transposed-scores):
  * Scores computed in NATURAL layout [q, keys]: 4 chained 512-col matmuls
    per 4-key-tile group (minimal PE instruction count).
  * Row-sums l come FREE from the Exp activation's accum_out (sums along
    the free/key axis in natural layout) -- no ones-matmuls.
  * Causality on the last 8 key tiles enforced with an ADDITIVE bf16 mask
    (0 / -32768) on the PSUM scores before exp (exp underflows to 0).
  * p is transposed on PE (4 identity-matmuls per group) into pT, which
    feeds the same z accumulation as kernel2.
  * Finalize is stage-split across the next q-tile's groups.
"""

import sys
from contextlib import ExitStack, nullcontext

if "/opt/trn_rl_repo" not in sys.path:
    sys.path.insert(0, "/opt/trn_rl_repo")

import numpy as np
import ml_dtypes

import concourse.bass as bass
import concourse.tile as tile
from concourse import bacc, mybir
from concourse.bass_utils import run_bass_kernel_spmd
from concourse.masks import make_identity

F32 = mybir.dt.float32
BF16 = mybir.dt.bfloat16
NPBF = ml_dtypes.bfloat16

N, D, NCORES = 8192, 512, 8
P = 128
NT = 8                 # q-tiles per core
DC = D // P            # 4 d-chunks
KT = N // P            # 64 key tiles total
MASKVAL = -32768.0     # additive causal mask; exp() underflows to exactly 0


def build(reps=1, trace_sim=False, stage="full", sbufs=2, tbufs=2, tlag=2,
          lacc=False):
    q_rows = NT * P
    nc = bacc.Bacc("TRN2", target_bir_lowering=False, debug=False,
                   num_devices=NCORES)
    xn_d = nc.dram_tensor("xn", [N, D], BF16, kind="ExternalInput").ap()
    xt_d = nc.dram_tensor("xt", [D, N], BF16, kind="ExternalInput").ap()
    xqt_d = nc.dram_tensor("xqt", [D, q_rows], BF16, kind="ExternalInput").ap()
    a_d = nc.dram_tensor("amat", [D, D], BF16, kind="ExternalInput").ap()
    wvt_d = nc.dram_tensor("wvt", [D, D], BF16, kind="ExternalInput").ap()
    v_d = nc.dram_tensor("vvec", [P, DC], F32, kind="ExternalInput").ap()
    mq_d = nc.dram_tensor("maskq", [P, NT * P], BF16, kind="ExternalInput").ap()
    mm_d = nc.dram_tensor("maskm", [P, NT * P], BF16, kind="ExternalInput").ap()
    out_d = nc.dram_tensor("out", [q_rows, D], F32, kind="ExternalOutput").ap()

    with tile.TileContext(nc, trace_sim=trace_sim) as tc, ExitStack() as st:
        consts = st.enter_context(tc.tile_pool(name="consts", bufs=1))
        big = st.enter_context(tc.tile_pool(name="big", bufs=1))
        la_p = st.enter_context(tc.tile_pool(name="lap", bufs=2))
        pn_p = st.enter_context(tc.tile_pool(name="pnp", bufs=4))
        pt_p = st.enter_context(tc.tile_pool(name="ptp", bufs=3))
        zn_p = st.enter_context(tc.tile_pool(name="znp", bufs=2))
        out_p = st.enter_context(tc.tile_pool(name="outp", bufs=2))
        ps_s = st.enter_context(tc.tile_pool(name="ps_s", bufs=sbufs, space="PSUM"))
        ps_t = st.enter_context(tc.tile_pool(name="ps_t", bufs=tbufs, space="PSUM"))
        ps_z = st.enter_context(tc.tile_pool(name="ps_z", bufs=2, space="PSUM"))
        ps_fo = st.enter_context(tc.tile_pool(name="ps_fo", bufs=2, space="PSUM"))

        loop = tc.For_i(0, reps, 1) if reps > 1 else nullcontext()
        with loop:
            ident = consts.tile([P, P], F32, tag="ident")
            make_identity(nc, ident)
            ident_b = consts.tile([P, P], BF16, tag="ident_b")
            nc.vector.tensor_copy(out=ident_b, in_=ident)

            a_sb = consts.tile([P, DC, D], BF16, tag="a_sb")
            nc.gpsimd.dma_start(out=a_sb,
                                in_=a_d.rearrange("(c p) d -> p c d", p=P))
            xqt_sb = consts.tile([P, DC, q_rows], BF16, tag="xqt")
            for qh in range(2):
                nc.gpsimd.dma_start(
                    out=xqt_sb[:, :, qh * 512:(qh + 1) * 512],
                    in_=xqt_d[:, qh * 512:(qh + 1) * 512]
                        .rearrange("(c p) n -> p c n", p=P))
            v_sb = consts.tile([P, DC], F32, tag="v_sb")
            nc.gpsimd.dma_start(out=v_sb, in_=v_d)
            mq_sb = consts.tile([P, NT * P], BF16, tag="mq")
            nc.gpsimd.dma_start(out=mq_sb, in_=mq_d if lacc else mm_d)

            # x resident in both layouts; 16 chunks, xt slice before xn slice
            # (scores consume xt earlier than z consumes xn).
            xn_sb = big.tile([P, KT, D], BF16, tag="xn")
            xt_sb = big.tile([P, DC, N], BF16, tag="xt")
            wvt_sb = consts.tile([P, DC, D], BF16, tag="wvt")
            for b in range(16):
                r0 = b * 512
                nc.gpsimd.dma_start(
                    out=xt_sb[:, :, r0:r0 + 512],
                    in_=xt_d[:, r0:r0 + 512].rearrange("(c p) n -> p c n", p=P))
                if stage != "sx":
                    nc.gpsimd.dma_start(
                        out=xn_sb[:, 4 * b:4 * b + 4, :],
                        in_=xn_d[r0:r0 + 512, :]
                            .rearrange("(t p) d -> p t d", p=P))
                if b == 0:
                    nc.gpsimd.dma_start(
                        out=wvt_sb,
                        in_=wvt_d.rearrange("(c p) d -> p c d", p=P))

            # ---- yT[dch*P+p, q] = (A.T @ xq.T)[d, q] + v[d] ----
            yT = consts.tile([P, DC, q_rows], BF16, tag="yT")
            for qh in range(q_rows // 512):
                for dch in range(DC):
                    ps = ps_fo.tile([P, 512], F32, tag="ps_fo")
                    for c in range(DC):
                        nc.tensor.matmul(ps,
                                         a_sb[:, c, dch * P:(dch + 1) * P],
                                         xqt_sb[:, c, qh * 512:(qh + 1) * 512],
                                         start=(c == 0), stop=(c == DC - 1))
                    nc.vector.tensor_scalar_add(
                        out=yT[:, dch, qh * 512:(qh + 1) * 512], in0=ps,
                        scalar1=v_sb[:, dch:dch + 1])

            linv = consts.tile([P, NT], F32, tag="linv")

            def fin_a(t, psz, la, G):
                lsum = out_p.tile([P, 1], F32, tag="lsum")
                nc.vector.reduce_sum(lsum, la[:, :G], axis=mybir.AxisListType.X)
                nc.vector.reciprocal(linv[:, t:t + 1], lsum)
                zn = zn_p.tile([P, D], F32, tag="zn")
                nc.vector.tensor_copy(out=zn, in_=psz)
                return zn

            def fin_b(zn):
                psf = ps_fo.tile([P, 512], F32, tag="ps_fo")
                for ch in range(DC):
                    nc.tensor.transpose(psf[:, ch * P:(ch + 1) * P],
                                        zn[:, ch * P:(ch + 1) * P], ident)
                return psf

            def fin_c(psf):
                znT = zn_p.tile([P, D], BF16, tag="znT")
                nc.vector.tensor_copy(out=znT, in_=psf)
                return znT

            def fin_d(t, znT):
                pso = ps_fo.tile([P, 512], F32, tag="ps_fo")
                for ch in range(DC):
                    nc.tensor.matmul(pso, znT[:, ch * P:(ch + 1) * P],
                                     wvt_sb[:, ch, :],
                                     start=(ch == 0), stop=(ch == DC - 1))
                ot = out_p.tile([P, D], F32, tag="outp")
                nc.vector.tensor_scalar_mul(out=ot, in0=pso,
                                            scalar1=linv[:, t:t + 1])
                nc.gpsimd.dma_start(out=out_d[t * P:(t + 1) * P, :], in_=ot)

            # pending finalize state machine: list of (stage_idx, t, payload)
            pending = []

            def pump_fin():
                if not pending:
                    return
                st_i, ft, payload = pending[0]
                if st_i == 0:
                    pending[0] = (1, ft, fin_a(ft, *payload))
                elif st_i == 1:
                    pending[0] = (2, ft, fin_b(payload))
                elif st_i == 2:
                    pending[0] = (3, ft, fin_c(payload))
                else:
                    fin_d(ft, payload)
                    pending.pop(0)

            for t in range(NT):
                Kt = 8 * (t + 1)
                G = Kt // 4
                psz = ps_z.tile([P, D], F32, tag="ps_z")
                la = la_p.tile([P, 16], F32, tag="la")
                tq = []   # (i, pt_nat) awaiting transpose
                zq = []   # (i, pT_sb) awaiting z-matmuls
                fin_budget = 4
                for i in range(G):
                    pss = ps_s.tile([P, 512], F32, tag="ps_s")
                    for c in range(DC):
                        nc.tensor.matmul(pss,
                                         yT[:, c, t * P:(t + 1) * P],
                                         xt_sb[:, c, 4 * i * P:(4 * i + 4) * P],
                                         start=(c == 0), stop=(c == DC - 1))
                    if lacc and i >= G - 2:
                        mh = i - (G - 2)
                        nc.vector.tensor_add(
                            out=pss, in0=pss,
                            in1=mq_sb[:, mh * 512:(mh + 1) * 512])
                    if stage in ("s", "sx"):
                        continue
                    pn = pn_p.tile([P, 512], BF16, tag="pn")
                    if lacc:
                        nc.scalar.activation(
                            out=pn, in_=pss,
                            func=mybir.ActivationFunctionType.Exp,
                            accum_out=la[:, i:i + 1])
                    else:
                        nc.scalar.activation(
                            out=pn, in_=pss,
                            func=mybir.ActivationFunctionType.Exp)
                        if i >= G - 2:
                            mh = i - (G - 2)
                            nc.vector.tensor_mul(
                                out=pn, in0=pn,
                                in1=mq_sb[:, mh * 512:(mh + 1) * 512])
                        nc.vector.reduce_sum(la[:, i:i + 1], pn,
                                             axis=mybir.AxisListType.X)
                    if stage == "se":
                        continue
                    tq.append((i, pn))
                    # lag-1: transpose group i-1 while scores of i stream
                    if len(tq) > tlag:
                        ti, tpn = tq.pop(0)
                        pst = ps_t.tile([P, 512], BF16, tag="ps_t")
                        for j in range(4):
                            nc.tensor.transpose(pst[:, j * P:(j + 1) * P],
                                                tpn[:, j * P:(j + 1) * P],
                                                ident_b)
                        ptb = pt_p.tile([P, 512], BF16, tag="pt")
                        nc.vector.tensor_copy(out=ptb, in_=pst)
                        zq.append((ti, ptb))
                    if len(zq) > 1:
                        zi, zpt = zq.pop(0)
                        for j in range(4):
                            kt = 4 * zi + j
                            nc.tensor.matmul(psz, zpt[:, j * P:(j + 1) * P],
                                             xn_sb[:, kt, :],
                                             start=(kt == 0),
                                             stop=(kt == Kt - 1))
                    if fin_budget > 0 and i >= 1:
                        pump_fin()
                        fin_budget -= 1
                if stage in ("s", "sx", "se"):
                    continue
                # drain
                while tq:
                    ti, tpn = tq.pop(0)
                    pst = ps_t.tile([P, 512], BF16, tag="ps_t")
                    for j in range(4):
                        nc.tensor.transpose(pst[:, j * P:(j + 1) * P],
                                            tpn[:, j * P:(j + 1) * P], ident_b)
                    ptb = pt_p.tile([P, 512], BF16, tag="pt")
                    nc.vector.tensor_copy(out=ptb, in_=pst)
                    zq.append((ti, ptb))
                while zq:
                    zi, zpt = zq.pop(0)
                    for j in range(4):
                        kt = 4 * zi + j
                        nc.tensor.matmul(psz, zpt[:, j * P:(j + 1) * P],
                                         xn_sb[:, kt, :],
                                         start=(kt == 0), stop=(kt == Kt - 1))
                if stage == "full":
                    pending.append((0, t, (psz, la, G)))
            if stage == "full":
                while pending:
                    pump_fin()

    nc.compile()
    return nc


def core_rows(c):
    tiles = list(range(c, KT, NCORES))
    return np.concatenate([np.arange(g * P, (g + 1) * P) for g in tiles])


def _check_causal_mask(mask):
    m = np.asarray(mask)
    assert m.shape == (N, N), f"mask shape {m.shape}"
    rng = np.random.default_rng(0)
    rows = rng.choice(N, size=64, replace=False)
    cols = np.arange(N)
    sub = m[rows]
    expect = np.where(cols[None, :] <= rows[:, None], 0.0, -1e9).astype(np.float32)
    if not np.array_equal(sub, expect):
        raise ValueError("mask is not the expected causal mask; "
                         "this kernel hardcodes causal structure")


def prepare_in_maps(x, mask, Wq, bq, Wk, bk, Wv, bv):
    x = np.asarray(x, dtype=np.float32)
    _check_causal_mask(mask)
    inv_sqrt_d = 1.0 / np.sqrt(D)
    A = (np.asarray(Wq).T.astype(np.float64) @ np.asarray(Wk).astype(np.float64)
         * inv_sqrt_d).astype(np.float32)
    wvT = np.ascontiguousarray(np.asarray(Wv).T)
    vvec = (np.asarray(Wk).T @ np.asarray(bq) * inv_sqrt_d).astype(np.float32)
    vvec = np.ascontiguousarray(vvec.reshape(DC, P).T)  # [P, DC]
    xn_b = x.astype(NPBF)
    xt_b = np.ascontiguousarray(x.T).astype(NPBF)
    a_b = A.astype(NPBF)
    wvt_b = wvT.astype(NPBF)

    qp = np.arange(P)[:, None, None]
    kl = np.arange(NT)[None, :, None]
    kp = np.arange(P)[None, None, :]
    rows = [core_rows(c) for c in range(NCORES)]
    in_maps = []
    for c in range(NCORES):
        live = (kl * P + kp <= c * P + qp)           # [qp, kl, kp]
        mq = np.where(live, 0.0, MASKVAL).astype(NPBF).reshape(P, NT * P)
        mm = live.astype(NPBF).reshape(P, NT * P)
        xqt = np.ascontiguousarray(x[rows[c]].T).astype(NPBF)
        in_maps.append({
            "xn": xn_b, "xt": xt_b, "xqt": xqt, "amat": a_b,
            "wvt": wvt_b, "vvec": vvec,
            "maskq": np.ascontiguousarray(mq),
            "maskm": np.ascontiguousarray(mm),
        })
    meta = {"rows": rows, "bv": np.asarray(bv, dtype=np.float32)}
    return in_maps, meta


_CACHED = {}


def kernel(x, mask, Wq, bq, Wk, bk, Wv, bv):
    x = np.asarray(x)
    in_maps, meta = prepare_in_maps(x, mask, Wq, bq, Wk, bk, Wv, bv)
    if "nc" not in _CACHED:
        _CACHED["nc"] = build()
    nc = _CACHED["nc"]
    res = run_bass_kernel_spmd(nc, in_maps, list(range(NCORES)))
    out = np.empty((N, D), np.float32)
    for c in range(NCORES):
        out[meta["rows"][c]] = res.results[c]["out"]
    out += meta["bv"][None, :]
    return out
